# revision 12
# baseline (speedup 1.0000x reference)
"""Trainium2 Bass kernel for nn_DGCRM_88227218194820.

The reference module's dynamic-adjacency branch (gconv_hyper / nodevec /
adp) is dead code w.r.t. the returned hidden state: due to the faithful
source bug, gconv_rnn(inp, i) == concat([inp, a*inp, a*inp], -1) @ rnn_W[i]
+ rnn_b[i] uses no adjacency, and the normalized adjacencies are deleted.
The output therefore reduces to a per-row GRU gate:

    combined = concat(x, h)                      # [.., 66]
    z  = sigmoid(combined @ Wz + bz)
    r  = sigmoid(combined @ Wr + br)
    hc = tanh(concat(x, r*h) @ Wc + bc)
    out = z*h + (1-z)*hc

with Wg folded from rnn_W: Wg = W[:66] + a*(W[66:132] + W[132:198]),
summed over the two gconv_rnn calls per gate.

Layout (per core, data-parallel over batch: 2 of 16 batches per core,
R = 2048 rows): everything lives transposed (channels on partitions) and
"group-stacked" -- rows 0:1024 (group A) on partitions 0:64, rows
1024:2048 (group B) on partitions 64:128, so every ACT/DVE op uses all
128 partitions.  Each gate matmul uses a K=128 block-diagonal bf16
weight blockdiag(Wg_h, Wg_h); the 2-channel x contribution AND the gate
bias (as a constant-1 input channel) accumulate via a K=6 block-diagonal
matmul.

v3 perf notes (HBM->SBUF DMA here is PACKET-latency bound: one packet
per SBUF partition row at ~13-20ns pumped by a shared engine pool, so
h+weights ride ONE 128-descriptor transfer; splitting it across queues
moves MORE packets and is slower -- measured):
 - auxh (h^T + K=128 weights, 128x2816B) alone on the sync HW queue;
   aux2 (x data + x/bias weights, 6 packets) on the scalar HW queue so
   neither stalls the other (the gpsimd software queue stalls the Pool
   engine ~4us; scalar drags a 1.3us table load ahead of its desc-gen
   but still lands aux2 ~1.2us before auxh).  aux2 lands early enough
   that the K=6 x-projections run first.
 - PE order: r x-mms, r h-mms (sigmoid r is the critical path to the
   candidate matmul), z x-mms, z h-mms, c x-mms, c h-mms (need r*h).
 - ACT chain sigr[1024], sigz[1024], tanh 2x[512] ~3.6us is the
   compute floor; all elementwise stays on the DVE (a GpSimd offload
   measured 3x slower AND slowed the concurrent DVE op via SBUF port
   contention on the shared source tiles).
 - ONE output DMA [128,2048B] post-context on sync, fire-and-forget.
 - trailing dummy matmuls / tiny activations on Tensor/Scalar (capped
   at the Vector tail's finish time) probe whether the walrus
   semaphore-sweep postamble (the 6.4us tail that bounds program end;
   ~125ns/op on the Tensor sequencer) runs at the HAM-gated clock.
"""

import ml_dtypes
import numpy as np

import concourse.tile as tile
from concourse import bacc, mybir
from concourse.bass_utils import run_bass_kernel_spmd

N_CORES = 8
B, N, IN_DIM, HID = 16, 1024, 2, 64
GC_ALPHA = 0.05
CIN = HID + IN_DIM          # 66
R = (B // N_CORES) * N      # 2048 rows per core
G = R // 2                  # 1024 rows per group (A/B)
BLK = 512                   # psum free-dim block
N_WARMUP_MM = 12
WARM_COLS = 256

F32 = mybir.dt.float32
BF16 = mybir.dt.bfloat16
AF = mybir.ActivationFunctionType
BF16_NP = ml_dtypes.bfloat16

_program_cache = {}


def build_program():
    nc = bacc.Bacc()
    # auxh: full h^T (bf16) + blockdiag gate weights (bf16), bitcast-
    # packed as ONE f32 transfer: 128 descriptors of 2816B (descriptor-
    # latency bound; do not split).
    auxh = nc.dram_tensor("auxh", [128, 704], F32, kind="ExternalInput")
    # aux2: bf16 blockdiag x+bias weights and x+ones data, bitcast-packed
    aux2 = nc.dram_tensor("aux2", [6, 704], F32, kind="ExternalInput")
    ot = nc.dram_tensor("ot", [128, G], BF16, kind="ExternalOutput")
    # Raw (non-tile) SBUF tensor so its concrete AP can feed the post-
    # context fire-and-forget output DMA.
    OT = nc.alloc_sbuf_tensor("OT", [128, G], BF16)

    with tile.TileContext(nc) as tc:
        with (
            tc.tile_pool(name="sb", bufs=1) as sb,
            tc.tile_pool(name="ps", bufs=1, space="PSUM") as ps,
        ):
            AUXH = sb.tile([128, 704], F32, tag="AUXH")
            AUX2 = sb.tile([6, 704], F32, tag="AUX2")
            ZT = sb.tile([128, G], BF16, tag="ZT")
            RT = sb.tile([128, G], BF16, tag="RT")
            RHB = sb.tile([128, G], BF16, tag="RHB")
            HC = sb.tile([128, G], BF16, tag="HC")
            OZ = sb.tile([128, G], BF16, tag="OZ")
            ZH = sb.tile([128, G], BF16, tag="ZH")
            MC = sb.tile([128, G], BF16, tag="MC")
            WARM = sb.tile([128, WARM_COLS], BF16, tag="WARM")
            dummy = sb.tile([1, 1], F32, tag="dummy")

            HTB0 = AUXH[:, 0:256].bitcast(BF16)    # [128, 512] h^T 0:512
            HTB1 = AUXH[:, 256:512].bitcast(BF16)  # [128, 512] h^T 512:1024
            WB = AUXH[:, 512:704].bitcast(BF16)    # [128, 384]
            WX = AUX2[:, 0:192].bitcast(BF16)      # [6, 384]
            XT = AUX2[:, 192:704].bitcast(BF16)    # [6, 1024]

            # Input DMAs first: auxh alone on the sync queue, aux2 alone
            # on the scalar queue.  The scalar desc-gen must precede the
            # (bacc-inserted) ACT table load in the scalar stream, so the
            # table-load trigger (dummy activation) is emitted later.
            with tc.high_priority():
                nc.sync.dma_start(out=AUXH, in_=auxh[:, :])
                nc.scalar.dma_start(out=AUX2, in_=aux2[:, :])

            nc.vector.memset(WARM, 0.0)
            nc.vector.memset(dummy, 0.0)

            def mm_h(psum_t, g, rhs_t, cols, n=BLK, start=False, stop=True):
                wc = slice(128 * g, 128 * g + 128)
                nc.tensor.matmul(
                    psum_t[:, 0:n], WB[:, wc], rhs_t[:, cols],
                    start=start, stop=stop, skip_group_check=True,
                )

            def mm_xb(psum_t, g, cols, n=BLK, start=True, stop=False):
                # x channels + constant-1 bias channel, K=6 blockdiag
                wc = slice(128 * g, 128 * g + 128)
                nc.tensor.matmul(
                    psum_t[:, 0:n], WX[0:6, wc], XT[0:6, cols],
                    start=start, stop=stop, skip_group_check=True,
                )

            cols0 = slice(0, BLK)
            cols1 = slice(BLK, G)
            colsL = slice(0, BLK)  # local cols within second-half tile
            pr = ps.tile([128, G], F32, tag="pr")
            pz = ps.tile([128, G], F32, tag="pz")
            pc0 = ps.tile([128, BLK], F32, tag="pc0")
            pc1 = ps.tile([128, BLK], F32, tag="pc1")
            pwarm = ps.tile([128, WARM_COLS], F32, tag="pwarm")

            for _ in range(N_WARMUP_MM):
                nc.tensor.matmul(
                    pwarm[:, :], WARM[:, 0:128], WARM[:, :],
                    start=True, stop=True, skip_group_check=True,
                )

            # Fire the ACT table load (sigmoid_and_others, covers tanh):
            # emitted after the DMAs/warmups so the hoisted table load
            # lands behind the scalar desc-gen, overlapping the DMAs.
            nc.scalar.activation(
                out=dummy, in_=dummy, func=AF.Sigmoid, bias=dummy[0:1, 0:1]
            )

            # ---- flattened schedule ----
            # r gate first: x-projections (early aux2) open the psum
            # groups, h matmuls close them the moment auxh lands.
            mm_xb(pr[:, cols0], 1, cols0, start=True, stop=False)
            mm_xb(pr[:, cols1], 1, cols1, start=True, stop=False)
            mm_h(pr[:, cols0], 1, HTB0, cols0, start=False, stop=True)
            mm_h(pr[:, cols1], 1, HTB1, colsL, start=False, stop=True)
            nc.scalar.activation(out=RT[:, :], in_=pr[:, :], func=AF.Sigmoid)
            nc.vector.tensor_mul(RHB[:, cols0], RT[:, cols0], HTB0[:, :])
            nc.vector.tensor_mul(RHB[:, cols1], RT[:, cols1], HTB1[:, :])

            # z sigmoid split in two 512-col halves so the DVE can start
            # oz/zh on half 0 while half 1 is still in the ACT pipe.
            mm_xb(pz[:, cols0], 0, cols0, start=True, stop=False)
            mm_xb(pz[:, cols1], 0, cols1, start=True, stop=False)
            mm_h(pz[:, cols0], 0, HTB0, cols0, start=False, stop=True)
            mm_h(pz[:, cols1], 0, HTB1, colsL, start=False, stop=True)
            nc.scalar.activation(
                out=ZT[:, cols0], in_=pz[:, cols0], func=AF.Sigmoid
            )
            nc.scalar.activation(
                out=ZT[:, cols1], in_=pz[:, cols1], func=AF.Sigmoid
            )

            mm_xb(pc0, 2, cols0)
            mm_xb(pc1, 2, cols1)
            mm_h(pc0, 2, RHB, cols0)
            mm_h(pc1, 2, RHB, cols1)

            # While the c matmuls run: oz = 1-z, zh = z*h; after each
            # tanh only mc/ot remain per block: out = zh + oz*hc.
            nc.vector.tensor_scalar(
                OZ[:, cols0], ZT[:, cols0], -1.0, 1.0,
                op0=mybir.AluOpType.mult, op1=mybir.AluOpType.add,
            )
            nc.vector.tensor_mul(ZH[:, cols0], ZT[:, cols0], HTB0[:, :])
            nc.vector.tensor_scalar(
                OZ[:, cols1], ZT[:, cols1], -1.0, 1.0,
                op0=mybir.AluOpType.mult, op1=mybir.AluOpType.add,
            )
            nc.vector.tensor_mul(ZH[:, cols1], ZT[:, cols1], HTB1[:, :])

            nc.scalar.activation(out=HC[:, cols0], in_=pc0[:, :], func=AF.Tanh)
            nc.scalar.activation(out=HC[:, cols1], in_=pc1[:, :], func=AF.Tanh)

            nc.vector.tensor_mul(MC[:, cols0], OZ[:, cols0], HC[:, cols0])
            nc.vector.tensor_add(OT[:, cols0], ZH[:, cols0], MC[:, cols0])
            nc.vector.tensor_mul(MC[:, cols1], OZ[:, cols1], HC[:, cols1])
            nc.vector.tensor_add(OT[:, cols1], ZH[:, cols1], MC[:, cols1])

    # Fire-and-forget output DMA, emitted AFTER the tile context: the
    # tile-exit all-engine barrier guarantees the blends are done, and
    # nothing waits on the transfer receipt -- it completes during the
    # compiler postamble.  (walrus requires sync info on DGE DMAs; osem
    # has no waiters.)
    osem = nc.alloc_semaphore("osem")
    nc.sync.dma_start(out=ot[:, :], in_=OT[:, :]).then_inc(osem, 16)

    nc.compile()
    return nc


def get_program():
    if "nc" not in _program_cache:
        _program_cache["nc"] = build_program()
    return _program_cache["nc"]


def fold_params(rnn_W, rnn_b):
    """Fold the gconv_rnn bug + gate sums into per-gate [66,64] weights."""
    Wf = rnn_W[:, :CIN, :] + GC_ALPHA * (
        rnn_W[:, CIN : 2 * CIN, :] + rnn_W[:, 2 * CIN : 3 * CIN, :]
    )  # [6, 66, 64]
    Wg = np.stack([Wf[0] + Wf[1], Wf[2] + Wf[3], Wf[4] + Wf[5]])  # [3,66,64]
    bg = np.stack(
        [rnn_b[0] + rnn_b[1], rnn_b[2] + rnn_b[3], rnn_b[4] + rnn_b[5]]
    )  # [3, 64]
    return Wg, bg


def make_in_maps(x, h, rnn_W, rnn_b):
    Wg, bg = fold_params(rnn_W, rnn_b)
    # combined = concat(x, h): channels 0:2 are x, 2:66 are h.
    # Gate order in the packed weights: z=0, r=1, c=2.
    W_x = Wg[:, :IN_DIM, :]  # [3, 2, 64]
    W_h = Wg[:, IN_DIM:, :]  # [3, 64, 64]

    # Block-diagonal bf16 weights: gate g occupies cols 128g:128(g+1);
    # out = blockdiag(Wg_h, Wg_h).T @ [h_A; h_B] = [gate_A; gate_B].
    # wx rows per group: [x0; x1; 1] -> [Wg_x; bg] folds the bias in.
    wb_host = np.zeros((128, 384), BF16_NP)
    wx_host = np.zeros((6, 384), BF16_NP)
    for g in range(3):
        wb_host[0:64, 128 * g : 128 * g + 64] = W_h[g]
        wb_host[64:128, 128 * g + 64 : 128 * g + 128] = W_h[g]
        wx_host[0:2, 128 * g : 128 * g + 64] = W_x[g]
        wx_host[2, 128 * g : 128 * g + 64] = bg[g]
        wx_host[3:5, 128 * g + 64 : 128 * g + 128] = W_x[g]
        wx_host[5, 128 * g + 64 : 128 * g + 128] = bg[g]

    hf = h.reshape(N_CORES, R, HID)
    xf = x.reshape(N_CORES, R, IN_DIM)
    in_maps = []
    for c in range(N_CORES):
        ht_host = np.ascontiguousarray(
            np.concatenate([hf[c, :G].T, hf[c, G:].T], axis=0)
        ).astype(BF16_NP)  # [128, G] bf16
        auxh_host = np.empty((128, 704), np.float32)
        auxh_host[:, 0:512] = ht_host.view(np.float32)
        auxh_host[:, 512:704] = wb_host.view(np.float32)
        xt_host = np.empty((6, G), BF16_NP)
        xt_host[0:2] = xf[c, :G].T
        xt_host[2] = 1.0
        xt_host[3:5] = xf[c, G:].T
        xt_host[5] = 1.0
        aux2_host = np.empty((6, 704), np.float32)
        aux2_host[:, 0:192] = wx_host.view(np.float32)
        aux2_host[:, 192:704] = xt_host.view(np.float32)
        in_maps.append(dict(auxh=auxh_host, aux2=aux2_host))
    return in_maps


def gather_output(results):
    outs = []
    for c in range(N_CORES):
        o = np.asarray(results[c]["ot"]).astype(np.float32)  # [128, G]
        outs.append(np.concatenate([o[:64].T, o[64:].T], axis=0))  # [R, HID]
    return (
        np.concatenate(outs, axis=0).reshape(B, N, HID).astype(np.float32)
    )


def run(inputs, trace=False, **kw):
    x = np.ascontiguousarray(np.asarray(inputs["x"], dtype=np.float32))
    h = np.ascontiguousarray(
        np.asarray(inputs["hidden_state"], dtype=np.float32)
    )
    rnn_W = np.asarray(inputs["rnn_W"], dtype=np.float32)
    rnn_b = np.asarray(inputs["rnn_b"], dtype=np.float32)

    in_maps = make_in_maps(x, h, rnn_W, rnn_b)
    nc = get_program()
    res = run_bass_kernel_spmd(
        nc, in_maps, core_ids=list(range(N_CORES)), trace=trace, **kw
    )
    return gather_output(res.results), res


def kernel(**inputs) -> np.ndarray:
    out, _ = run(inputs)
    return out


# revision 13
# speedup vs baseline: 1.1386x; 1.1386x over previous
"""Trainium2 Bass kernel for nn_DGCRM_88227218194820.

The reference module's dynamic-adjacency branch (gconv_hyper / nodevec /
adp) is dead code w.r.t. the returned hidden state: due to the faithful
source bug, gconv_rnn(inp, i) == concat([inp, a*inp, a*inp], -1) @ rnn_W[i]
+ rnn_b[i] uses no adjacency, and the normalized adjacencies are deleted.
The output therefore reduces to a per-row GRU gate:

    combined = concat(x, h)                      # [.., 66]
    z  = sigmoid(combined @ Wz + bz)
    r  = sigmoid(combined @ Wr + br)
    hc = tanh(concat(x, r*h) @ Wc + bc)
    out = z*h + (1-z)*hc

with Wg folded from rnn_W: Wg = W[:66] + a*(W[66:132] + W[132:198]),
summed over the two gconv_rnn calls per gate.

Layout (per core, data-parallel over batch: 2 of 16 batches per core,
R = 2048 rows): everything lives transposed (channels on partitions) and
"group-stacked" -- rows 0:1024 (group A) on partitions 0:64, rows
1024:2048 (group B) on partitions 64:128, so every ACT/DVE op uses all
128 partitions.  Each gate matmul uses a K=128 block-diagonal bf16
weight blockdiag(Wg_h, Wg_h); the 2-channel x contribution AND the gate
bias (as a constant-1 input channel) accumulate via a K=6 block-diagonal
matmul.

v3 perf notes (HBM->SBUF DMA here is PACKET-latency bound: one packet
per SBUF partition row at ~13-20ns pumped by a shared engine pool, so
h+weights ride ONE 128-descriptor transfer; splitting it across queues
moves MORE packets and is slower -- measured):
 - auxh (h^T + K=128 weights, 128x2816B) alone on the sync HW queue;
   aux2 (x data + x/bias weights, 6 packets) on the scalar HW queue so
   neither stalls the other (the gpsimd software queue stalls the Pool
   engine ~4us; scalar drags a 1.3us table load ahead of its desc-gen
   but still lands aux2 ~1.2us before auxh).  aux2 lands early enough
   that the K=6 x-projections run first.
 - PE order: r x-mms, r h-mms (sigmoid r is the critical path to the
   candidate matmul), z x-mms, z h-mms, c x-mms, c h-mms (need r*h).
 - ACT chain sigr[1024], sigz[1024], tanh 2x[512] ~3.6us is the
   compute floor; all elementwise stays on the DVE (a GpSimd offload
   measured 3x slower AND slowed the concurrent DVE op via SBUF port
   contention on the shared source tiles).
 - ONE output DMA [128,2048B] post-context on sync, fire-and-forget.
 - trailing dummy matmuls / tiny activations on Tensor/Scalar (capped
   at the Vector tail's finish time) probe whether the walrus
   semaphore-sweep postamble (the 6.4us tail that bounds program end;
   ~125ns/op on the Tensor sequencer) runs at the HAM-gated clock.
"""

import ml_dtypes
import numpy as np

import concourse.tile as tile
from concourse import bacc, mybir
from concourse.bass_utils import run_bass_kernel_spmd

N_CORES = 8
B, N, IN_DIM, HID = 16, 1024, 2, 64
GC_ALPHA = 0.05
CIN = HID + IN_DIM          # 66
R = (B // N_CORES) * N      # 2048 rows per core
G = R // 2                  # 1024 rows per group (A/B)
BLK = 512                   # psum free-dim block
N_WARMUP_MM = 6
WARM_COLS = 256

F32 = mybir.dt.float32
BF16 = mybir.dt.bfloat16
AF = mybir.ActivationFunctionType
BF16_NP = ml_dtypes.bfloat16

_program_cache = {}


def build_program():
    nc = bacc.Bacc()
    # auxh: full h^T (bf16) + blockdiag gate weights (bf16), bitcast-
    # packed as ONE f32 transfer: 128 descriptors of 2816B (descriptor-
    # latency bound; do not split).
    auxh = nc.dram_tensor("auxh", [128, 704], F32, kind="ExternalInput")
    # aux2: bf16 blockdiag x+bias weights and x+ones data, bitcast-packed
    aux2 = nc.dram_tensor("aux2", [6, 704], F32, kind="ExternalInput")
    ot = nc.dram_tensor("ot", [128, G], BF16, kind="ExternalOutput")
    # Raw (non-tile) SBUF tensor so its concrete AP can feed the post-
    # context fire-and-forget output DMA.
    OT = nc.alloc_sbuf_tensor("OT", [128, G], BF16)

    with tile.TileContext(nc) as tc:
        with (
            tc.tile_pool(name="sb", bufs=1) as sb,
            tc.tile_pool(name="ps", bufs=1, space="PSUM") as ps,
        ):
            AUXH = sb.tile([128, 704], F32, tag="AUXH")
            AUX2 = sb.tile([6, 704], F32, tag="AUX2")
            ZT = sb.tile([128, G], BF16, tag="ZT")
            RT = sb.tile([128, G], BF16, tag="RT")
            RHB = sb.tile([128, G], BF16, tag="RHB")
            HC = sb.tile([128, G], BF16, tag="HC")
            OZ = sb.tile([128, G], BF16, tag="OZ")
            ZH = sb.tile([128, G], BF16, tag="ZH")
            MC = sb.tile([128, G], BF16, tag="MC")
            WARM = sb.tile([128, WARM_COLS], BF16, tag="WARM")
            dummy = sb.tile([1, 1], F32, tag="dummy")

            HTB0 = AUXH[:, 0:256].bitcast(BF16)    # [128, 512] h^T 0:512
            HTB1 = AUXH[:, 256:512].bitcast(BF16)  # [128, 512] h^T 512:1024
            WB = AUXH[:, 512:704].bitcast(BF16)    # [128, 384]
            WX = AUX2[:, 0:192].bitcast(BF16)      # [6, 384]
            XT = AUX2[:, 192:704].bitcast(BF16)    # [6, 1024]

            # Input DMAs first: auxh alone on the sync queue, aux2 alone
            # on the scalar queue.  The scalar desc-gen must precede the
            # (bacc-inserted) ACT table load in the scalar stream, so the
            # table-load trigger (dummy activation) is emitted later.
            with tc.high_priority():
                nc.sync.dma_start(out=AUXH, in_=auxh[:, :])
                nc.scalar.dma_start(out=AUX2, in_=aux2[:, :])

            nc.vector.memset(WARM, 0.0)
            nc.vector.memset(dummy, 0.0)

            def mm_h(psum_t, g, rhs_t, cols, n=BLK, start=False, stop=True):
                wc = slice(128 * g, 128 * g + 128)
                nc.tensor.matmul(
                    psum_t[:, 0:n], WB[:, wc], rhs_t[:, cols],
                    start=start, stop=stop, skip_group_check=True,
                )

            def mm_xb(psum_t, g, cols, n=BLK, start=True, stop=False):
                # x channels + constant-1 bias channel, K=6 blockdiag
                wc = slice(128 * g, 128 * g + 128)
                nc.tensor.matmul(
                    psum_t[:, 0:n], WX[0:6, wc], XT[0:6, cols],
                    start=start, stop=stop, skip_group_check=True,
                )

            cols0 = slice(0, BLK)
            cols1 = slice(BLK, G)
            colsL = slice(0, BLK)  # local cols within second-half tile
            pr = ps.tile([128, G], F32, tag="pr")
            pz = ps.tile([128, G], F32, tag="pz")
            pc0 = ps.tile([128, BLK], F32, tag="pc0")
            pc1 = ps.tile([128, BLK], F32, tag="pc1")
            pwarm = ps.tile([128, WARM_COLS], F32, tag="pwarm")

            for _ in range(N_WARMUP_MM):
                nc.tensor.matmul(
                    pwarm[:, :], WARM[:, 0:128], WARM[:, :],
                    start=True, stop=True, skip_group_check=True,
                )

            # Fire the ACT table load (sigmoid_and_others, covers tanh):
            # emitted after the DMAs/warmups so the hoisted table load
            # lands behind the scalar desc-gen, overlapping the DMAs.
            nc.scalar.activation(
                out=dummy, in_=dummy, func=AF.Sigmoid, bias=dummy[0:1, 0:1]
            )

            # ---- flattened schedule ----
            # r gate first: x-projections (early aux2) open the psum
            # groups, h matmuls close them the moment auxh lands.
            mm_xb(pr[:, cols0], 1, cols0, start=True, stop=False)
            mm_xb(pr[:, cols1], 1, cols1, start=True, stop=False)
            mm_h(pr[:, cols0], 1, HTB0, cols0, start=False, stop=True)
            mm_h(pr[:, cols1], 1, HTB1, colsL, start=False, stop=True)
            nc.scalar.activation(out=RT[:, :], in_=pr[:, :], func=AF.Sigmoid)
            nc.vector.tensor_mul(RHB[:, cols0], RT[:, cols0], HTB0[:, :])
            nc.vector.tensor_mul(RHB[:, cols1], RT[:, cols1], HTB1[:, :])

            # z sigmoid split in two 512-col halves so the DVE can start
            # oz/zh on half 0 while half 1 is still in the ACT pipe.
            mm_xb(pz[:, cols0], 0, cols0, start=True, stop=False)
            mm_xb(pz[:, cols1], 0, cols1, start=True, stop=False)
            mm_h(pz[:, cols0], 0, HTB0, cols0, start=False, stop=True)
            mm_h(pz[:, cols1], 0, HTB1, colsL, start=False, stop=True)
            nc.scalar.activation(
                out=ZT[:, cols0], in_=pz[:, cols0], func=AF.Sigmoid
            )
            nc.scalar.activation(
                out=ZT[:, cols1], in_=pz[:, cols1], func=AF.Sigmoid
            )

            mm_xb(pc0, 2, cols0)
            mm_xb(pc1, 2, cols1)
            mm_h(pc0, 2, RHB, cols0)
            mm_h(pc1, 2, RHB, cols1)

            # While the c matmuls run: oz = 1-z, zh = z*h; after each
            # tanh only mc/ot remain per block: out = zh + oz*hc.
            nc.vector.tensor_scalar(
                OZ[:, cols0], ZT[:, cols0], -1.0, 1.0,
                op0=mybir.AluOpType.mult, op1=mybir.AluOpType.add,
            )
            nc.vector.tensor_mul(ZH[:, cols0], ZT[:, cols0], HTB0[:, :])
            nc.vector.tensor_scalar(
                OZ[:, cols1], ZT[:, cols1], -1.0, 1.0,
                op0=mybir.AluOpType.mult, op1=mybir.AluOpType.add,
            )
            nc.vector.tensor_mul(ZH[:, cols1], ZT[:, cols1], HTB1[:, :])

            nc.scalar.activation(out=HC[:, cols0], in_=pc0[:, :], func=AF.Tanh)
            nc.scalar.activation(out=HC[:, cols1], in_=pc1[:, :], func=AF.Tanh)

            nc.vector.tensor_mul(MC[:, cols0], OZ[:, cols0], HC[:, cols0])
            nc.vector.tensor_add(OT[:, cols0], ZH[:, cols0], MC[:, cols0])
            nc.vector.tensor_mul(MC[:, cols1], OZ[:, cols1], HC[:, cols1])
            nc.vector.tensor_add(OT[:, cols1], ZH[:, cols1], MC[:, cols1])

    # Fire-and-forget output DMA, emitted AFTER the tile context: the
    # tile-exit all-engine barrier guarantees the blends are done, and
    # nothing waits on the transfer receipt -- it completes during the
    # compiler postamble.  (walrus requires sync info on DGE DMAs; osem
    # has no waiters.)
    osem = nc.alloc_semaphore("osem")
    nc.sync.dma_start(out=ot[:, :], in_=OT[:, :]).then_inc(osem, 16)

    nc.compile()
    return nc


def get_program():
    if "nc" not in _program_cache:
        _program_cache["nc"] = build_program()
    return _program_cache["nc"]


def fold_params(rnn_W, rnn_b):
    """Fold the gconv_rnn bug + gate sums into per-gate [66,64] weights."""
    Wf = rnn_W[:, :CIN, :] + GC_ALPHA * (
        rnn_W[:, CIN : 2 * CIN, :] + rnn_W[:, 2 * CIN : 3 * CIN, :]
    )  # [6, 66, 64]
    Wg = np.stack([Wf[0] + Wf[1], Wf[2] + Wf[3], Wf[4] + Wf[5]])  # [3,66,64]
    bg = np.stack(
        [rnn_b[0] + rnn_b[1], rnn_b[2] + rnn_b[3], rnn_b[4] + rnn_b[5]]
    )  # [3, 64]
    return Wg, bg


def make_in_maps(x, h, rnn_W, rnn_b):
    Wg, bg = fold_params(rnn_W, rnn_b)
    # combined = concat(x, h): channels 0:2 are x, 2:66 are h.
    # Gate order in the packed weights: z=0, r=1, c=2.
    W_x = Wg[:, :IN_DIM, :]  # [3, 2, 64]
    W_h = Wg[:, IN_DIM:, :]  # [3, 64, 64]

    # Block-diagonal bf16 weights: gate g occupies cols 128g:128(g+1);
    # out = blockdiag(Wg_h, Wg_h).T @ [h_A; h_B] = [gate_A; gate_B].
    # wx rows per group: [x0; x1; 1] -> [Wg_x; bg] folds the bias in.
    wb_host = np.zeros((128, 384), BF16_NP)
    wx_host = np.zeros((6, 384), BF16_NP)
    for g in range(3):
        wb_host[0:64, 128 * g : 128 * g + 64] = W_h[g]
        wb_host[64:128, 128 * g + 64 : 128 * g + 128] = W_h[g]
        wx_host[0:2, 128 * g : 128 * g + 64] = W_x[g]
        wx_host[2, 128 * g : 128 * g + 64] = bg[g]
        wx_host[3:5, 128 * g + 64 : 128 * g + 128] = W_x[g]
        wx_host[5, 128 * g + 64 : 128 * g + 128] = bg[g]

    hf = h.reshape(N_CORES, R, HID)
    xf = x.reshape(N_CORES, R, IN_DIM)
    in_maps = []
    for c in range(N_CORES):
        ht_host = np.ascontiguousarray(
            np.concatenate([hf[c, :G].T, hf[c, G:].T], axis=0)
        ).astype(BF16_NP)  # [128, G] bf16
        auxh_host = np.empty((128, 704), np.float32)
        auxh_host[:, 0:512] = ht_host.view(np.float32)
        auxh_host[:, 512:704] = wb_host.view(np.float32)
        xt_host = np.empty((6, G), BF16_NP)
        xt_host[0:2] = xf[c, :G].T
        xt_host[2] = 1.0
        xt_host[3:5] = xf[c, G:].T
        xt_host[5] = 1.0
        aux2_host = np.empty((6, 704), np.float32)
        aux2_host[:, 0:192] = wx_host.view(np.float32)
        aux2_host[:, 192:704] = xt_host.view(np.float32)
        in_maps.append(dict(auxh=auxh_host, aux2=aux2_host))
    return in_maps


def gather_output(results):
    outs = []
    for c in range(N_CORES):
        o = np.asarray(results[c]["ot"]).astype(np.float32)  # [128, G]
        outs.append(np.concatenate([o[:64].T, o[64:].T], axis=0))  # [R, HID]
    return (
        np.concatenate(outs, axis=0).reshape(B, N, HID).astype(np.float32)
    )


def run(inputs, trace=False, **kw):
    x = np.ascontiguousarray(np.asarray(inputs["x"], dtype=np.float32))
    h = np.ascontiguousarray(
        np.asarray(inputs["hidden_state"], dtype=np.float32)
    )
    rnn_W = np.asarray(inputs["rnn_W"], dtype=np.float32)
    rnn_b = np.asarray(inputs["rnn_b"], dtype=np.float32)

    in_maps = make_in_maps(x, h, rnn_W, rnn_b)
    nc = get_program()
    res = run_bass_kernel_spmd(
        nc, in_maps, core_ids=list(range(N_CORES)), trace=trace, **kw
    )
    return gather_output(res.results), res


def kernel(**inputs) -> np.ndarray:
    out, _ = run(inputs)
    return out


# revision 14
# speedup vs baseline: 1.1584x; 1.0173x over previous
"""Trainium2 Bass kernel for nn_DGCRM_88227218194820.

The reference module's dynamic-adjacency branch (gconv_hyper / nodevec /
adp) is dead code w.r.t. the returned hidden state: due to the faithful
source bug, gconv_rnn(inp, i) == concat([inp, a*inp, a*inp], -1) @ rnn_W[i]
+ rnn_b[i] uses no adjacency, and the normalized adjacencies are deleted.
The output therefore reduces to a per-row GRU gate:

    combined = concat(x, h)                      # [.., 66]
    z  = sigmoid(combined @ Wz + bz)
    r  = sigmoid(combined @ Wr + br)
    hc = tanh(concat(x, r*h) @ Wc + bc)
    out = z*h + (1-z)*hc

with Wg folded from rnn_W: Wg = W[:66] + a*(W[66:132] + W[132:198]),
summed over the two gconv_rnn calls per gate.

Layout (per core, data-parallel over batch: 2 of 16 batches per core,
R = 2048 rows): everything lives transposed (channels on partitions) and
"group-stacked" -- rows 0:1024 (group A) on partitions 0:64, rows
1024:2048 (group B) on partitions 64:128, so every ACT/DVE op uses all
128 partitions.  Each gate matmul uses a K=128 block-diagonal bf16
weight blockdiag(Wg_h, Wg_h); the 2-channel x contribution AND the gate
bias (as a constant-1 input channel) accumulate via a K=6 block-diagonal
matmul.

Final perf notes (HBM->SBUF DMA here is PACKET-latency bound: one packet
per SBUF partition row at ~13-20ns pumped by a shared engine pool, so
h+weights ride ONE 128-descriptor transfer; splitting it across queues
moves MORE packets and is slower -- measured):
 - auxh (h^T + K=128 weights, 128x2816B) alone on the sync HW queue;
   aux2 (x data + x/bias weights, 6 packets) on the scalar HW queue so
   neither stalls the other (the gpsimd software queue stalls the Pool
   engine ~4us; scalar drags a 1.3us table load ahead of its desc-gen
   but still lands aux2 ~1.2us before auxh).  aux2 lands early enough
   that the K=6 x-projections run first.
 - PE order: r x-mms, r h-mms (sigmoid r is the critical path to the
   candidate matmul), z x-mms, z h-mms, c x-mms, c h-mms (need r*h).
 - ACT chain sigr[1024], sigz[1024], tanh 2x[512] ~3.6us is the
   compute floor; all elementwise stays on the DVE (a GpSimd offload
   measured 3x slower AND slowed the concurrent DVE op via SBUF port
   contention on the shared source tiles).
 - ONE output DMA [128,2048B] post-context on sync, fire-and-forget.
 - measured fixed tail after the last blend: tile-exit double barrier
   + output desc-gen + walrus body-end barrier (~2.3us), then the
   walrus postamble sweeping the 256-entry semaphore file split across
   the 5 engines (Tensor: 51 ops x ~117ns = 6.0us, the critical
   sweep; NOT HAM-gated -- measured identical warm/cold), final
   barrier ~0.7us.  None of it is controllable from Bass.
 - run-to-run variance ~+/-1us from DMA queue-start latency (h lands
   rel +4.5..+6.2) plus occasional ~20% global SW-throttle runs.
"""

import ml_dtypes
import numpy as np

import concourse.tile as tile
from concourse import bacc, mybir
from concourse.bass_utils import run_bass_kernel_spmd

N_CORES = 8
B, N, IN_DIM, HID = 16, 1024, 2, 64
GC_ALPHA = 0.05
CIN = HID + IN_DIM          # 66
R = (B // N_CORES) * N      # 2048 rows per core
G = R // 2                  # 1024 rows per group (A/B)
BLK = 512                   # psum free-dim block
N_WARMUP_MM = 6
WARM_COLS = 256

F32 = mybir.dt.float32
BF16 = mybir.dt.bfloat16
AF = mybir.ActivationFunctionType
BF16_NP = ml_dtypes.bfloat16

_program_cache = {}


def build_program():
    nc = bacc.Bacc()
    # auxh: full h^T (bf16) + blockdiag gate weights (bf16), bitcast-
    # packed as ONE f32 transfer: 128 descriptors of 2816B (descriptor-
    # latency bound; do not split).
    auxh = nc.dram_tensor("auxh", [128, 704], F32, kind="ExternalInput")
    # aux2: bf16 blockdiag x+bias weights and x+ones data, bitcast-packed
    aux2 = nc.dram_tensor("aux2", [6, 704], F32, kind="ExternalInput")
    ot = nc.dram_tensor("ot", [128, G], BF16, kind="ExternalOutput")
    # Raw (non-tile) SBUF tensor so its concrete AP can feed the post-
    # context fire-and-forget output DMA.
    OT = nc.alloc_sbuf_tensor("OT", [128, G], BF16)

    with tile.TileContext(nc) as tc:
        with (
            tc.tile_pool(name="sb", bufs=1) as sb,
            tc.tile_pool(name="ps", bufs=1, space="PSUM") as ps,
        ):
            AUXH = sb.tile([128, 704], F32, tag="AUXH")
            AUX2 = sb.tile([6, 704], F32, tag="AUX2")
            ZT = sb.tile([128, G], BF16, tag="ZT")
            RT = sb.tile([128, G], BF16, tag="RT")
            RHB = sb.tile([128, G], BF16, tag="RHB")
            HC = sb.tile([128, G], BF16, tag="HC")
            OZ = sb.tile([128, G], BF16, tag="OZ")
            ZH = sb.tile([128, G], BF16, tag="ZH")
            MC = sb.tile([128, G], BF16, tag="MC")
            WARM = sb.tile([128, WARM_COLS], BF16, tag="WARM")
            dummy = sb.tile([1, 1], F32, tag="dummy")

            HTB0 = AUXH[:, 0:256].bitcast(BF16)    # [128, 512] h^T 0:512
            HTB1 = AUXH[:, 256:512].bitcast(BF16)  # [128, 512] h^T 512:1024
            WB = AUXH[:, 512:704].bitcast(BF16)    # [128, 384]
            WX = AUX2[:, 0:192].bitcast(BF16)      # [6, 384]
            XT = AUX2[:, 192:704].bitcast(BF16)    # [6, 1024]

            # Input DMAs first: auxh alone on the sync queue, aux2 alone
            # on the scalar queue.  The scalar desc-gen must precede the
            # (bacc-inserted) ACT table load in the scalar stream, so the
            # table-load trigger (dummy activation) is emitted later.
            with tc.high_priority():
                nc.sync.dma_start(out=AUXH, in_=auxh[:, :])
                nc.scalar.dma_start(out=AUX2, in_=aux2[:, :])

            nc.vector.memset(WARM, 0.0)
            nc.vector.memset(dummy, 0.0)

            def mm_h(psum_t, g, rhs_t, cols, n=BLK, start=False, stop=True):
                wc = slice(128 * g, 128 * g + 128)
                nc.tensor.matmul(
                    psum_t[:, 0:n], WB[:, wc], rhs_t[:, cols],
                    start=start, stop=stop, skip_group_check=True,
                )

            def mm_xb(psum_t, g, cols, n=BLK, start=True, stop=False):
                # x channels + constant-1 bias channel, K=6 blockdiag
                wc = slice(128 * g, 128 * g + 128)
                nc.tensor.matmul(
                    psum_t[:, 0:n], WX[0:6, wc], XT[0:6, cols],
                    start=start, stop=stop, skip_group_check=True,
                )

            cols0 = slice(0, BLK)
            cols1 = slice(BLK, G)
            colsL = slice(0, BLK)  # local cols within second-half tile
            pr = ps.tile([128, G], F32, tag="pr")
            pz = ps.tile([128, G], F32, tag="pz")
            pc0 = ps.tile([128, BLK], F32, tag="pc0")
            pc1 = ps.tile([128, BLK], F32, tag="pc1")
            pwarm = ps.tile([128, WARM_COLS], F32, tag="pwarm")

            for _ in range(N_WARMUP_MM):
                nc.tensor.matmul(
                    pwarm[:, :], WARM[:, 0:128], WARM[:, :],
                    start=True, stop=True, skip_group_check=True,
                )

            # Fire the ACT table load (sigmoid_and_others, covers tanh):
            # emitted after the DMAs/warmups so the hoisted table load
            # lands behind the scalar desc-gen, overlapping the DMAs.
            nc.scalar.activation(
                out=dummy, in_=dummy, func=AF.Sigmoid, bias=dummy[0:1, 0:1]
            )

            # ---- flattened schedule ----
            # r gate first: x-projections (early aux2) open the psum
            # groups, h matmuls close them the moment auxh lands.
            mm_xb(pr[:, cols0], 1, cols0, start=True, stop=False)
            mm_xb(pr[:, cols1], 1, cols1, start=True, stop=False)
            mm_h(pr[:, cols0], 1, HTB0, cols0, start=False, stop=True)
            mm_h(pr[:, cols1], 1, HTB1, colsL, start=False, stop=True)
            nc.scalar.activation(out=RT[:, :], in_=pr[:, :], func=AF.Sigmoid)
            nc.vector.tensor_mul(RHB[:, cols0], RT[:, cols0], HTB0[:, :])
            nc.vector.tensor_mul(RHB[:, cols1], RT[:, cols1], HTB1[:, :])

            # z sigmoid split in two 512-col halves so the DVE can start
            # oz/zh on half 0 while half 1 is still in the ACT pipe.
            mm_xb(pz[:, cols0], 0, cols0, start=True, stop=False)
            mm_xb(pz[:, cols1], 0, cols1, start=True, stop=False)
            mm_h(pz[:, cols0], 0, HTB0, cols0, start=False, stop=True)
            mm_h(pz[:, cols1], 0, HTB1, colsL, start=False, stop=True)
            nc.scalar.activation(
                out=ZT[:, cols0], in_=pz[:, cols0], func=AF.Sigmoid
            )
            nc.scalar.activation(
                out=ZT[:, cols1], in_=pz[:, cols1], func=AF.Sigmoid
            )

            mm_xb(pc0, 2, cols0)
            mm_xb(pc1, 2, cols1)
            mm_h(pc0, 2, RHB, cols0)
            mm_h(pc1, 2, RHB, cols1)

            # While the c matmuls run: oz = 1-z, zh = z*h; after each
            # tanh only mc/ot remain per block: out = zh + oz*hc.
            nc.vector.tensor_scalar(
                OZ[:, cols0], ZT[:, cols0], -1.0, 1.0,
                op0=mybir.AluOpType.mult, op1=mybir.AluOpType.add,
            )
            nc.vector.tensor_mul(ZH[:, cols0], ZT[:, cols0], HTB0[:, :])
            nc.vector.tensor_scalar(
                OZ[:, cols1], ZT[:, cols1], -1.0, 1.0,
                op0=mybir.AluOpType.mult, op1=mybir.AluOpType.add,
            )
            nc.vector.tensor_mul(ZH[:, cols1], ZT[:, cols1], HTB1[:, :])

            nc.scalar.activation(out=HC[:, cols0], in_=pc0[:, :], func=AF.Tanh)
            nc.scalar.activation(out=HC[:, cols1], in_=pc1[:, :], func=AF.Tanh)

            nc.vector.tensor_mul(MC[:, cols0], OZ[:, cols0], HC[:, cols0])
            nc.vector.tensor_add(OT[:, cols0], ZH[:, cols0], MC[:, cols0])
            nc.vector.tensor_mul(MC[:, cols1], OZ[:, cols1], HC[:, cols1])
            nc.vector.tensor_add(OT[:, cols1], ZH[:, cols1], MC[:, cols1])

    # Fire-and-forget output DMA, emitted AFTER the tile context: the
    # tile-exit all-engine barrier guarantees the blends are done, and
    # nothing waits on the transfer receipt -- it completes during the
    # compiler postamble.  (walrus requires sync info on DGE DMAs; osem
    # has no waiters.)
    osem = nc.alloc_semaphore("osem")
    nc.sync.dma_start(out=ot[:, :], in_=OT[:, :]).then_inc(osem, 16)

    nc.compile()
    return nc


def get_program():
    if "nc" not in _program_cache:
        _program_cache["nc"] = build_program()
    return _program_cache["nc"]


def fold_params(rnn_W, rnn_b):
    """Fold the gconv_rnn bug + gate sums into per-gate [66,64] weights."""
    Wf = rnn_W[:, :CIN, :] + GC_ALPHA * (
        rnn_W[:, CIN : 2 * CIN, :] + rnn_W[:, 2 * CIN : 3 * CIN, :]
    )  # [6, 66, 64]
    Wg = np.stack([Wf[0] + Wf[1], Wf[2] + Wf[3], Wf[4] + Wf[5]])  # [3,66,64]
    bg = np.stack(
        [rnn_b[0] + rnn_b[1], rnn_b[2] + rnn_b[3], rnn_b[4] + rnn_b[5]]
    )  # [3, 64]
    return Wg, bg


def make_in_maps(x, h, rnn_W, rnn_b):
    Wg, bg = fold_params(rnn_W, rnn_b)
    # combined = concat(x, h): channels 0:2 are x, 2:66 are h.
    # Gate order in the packed weights: z=0, r=1, c=2.
    W_x = Wg[:, :IN_DIM, :]  # [3, 2, 64]
    W_h = Wg[:, IN_DIM:, :]  # [3, 64, 64]

    # Block-diagonal bf16 weights: gate g occupies cols 128g:128(g+1);
    # out = blockdiag(Wg_h, Wg_h).T @ [h_A; h_B] = [gate_A; gate_B].
    # wx rows per group: [x0; x1; 1] -> [Wg_x; bg] folds the bias in.
    wb_host = np.zeros((128, 384), BF16_NP)
    wx_host = np.zeros((6, 384), BF16_NP)
    for g in range(3):
        wb_host[0:64, 128 * g : 128 * g + 64] = W_h[g]
        wb_host[64:128, 128 * g + 64 : 128 * g + 128] = W_h[g]
        wx_host[0:2, 128 * g : 128 * g + 64] = W_x[g]
        wx_host[2, 128 * g : 128 * g + 64] = bg[g]
        wx_host[3:5, 128 * g + 64 : 128 * g + 128] = W_x[g]
        wx_host[5, 128 * g + 64 : 128 * g + 128] = bg[g]

    hf = h.reshape(N_CORES, R, HID)
    xf = x.reshape(N_CORES, R, IN_DIM)
    in_maps = []
    for c in range(N_CORES):
        ht_host = np.ascontiguousarray(
            np.concatenate([hf[c, :G].T, hf[c, G:].T], axis=0)
        ).astype(BF16_NP)  # [128, G] bf16
        auxh_host = np.empty((128, 704), np.float32)
        auxh_host[:, 0:512] = ht_host.view(np.float32)
        auxh_host[:, 512:704] = wb_host.view(np.float32)
        xt_host = np.empty((6, G), BF16_NP)
        xt_host[0:2] = xf[c, :G].T
        xt_host[2] = 1.0
        xt_host[3:5] = xf[c, G:].T
        xt_host[5] = 1.0
        aux2_host = np.empty((6, 704), np.float32)
        aux2_host[:, 0:192] = wx_host.view(np.float32)
        aux2_host[:, 192:704] = xt_host.view(np.float32)
        in_maps.append(dict(auxh=auxh_host, aux2=aux2_host))
    return in_maps


def gather_output(results):
    outs = []
    for c in range(N_CORES):
        o = np.asarray(results[c]["ot"]).astype(np.float32)  # [128, G]
        outs.append(np.concatenate([o[:64].T, o[64:].T], axis=0))  # [R, HID]
    return (
        np.concatenate(outs, axis=0).reshape(B, N, HID).astype(np.float32)
    )


def run(inputs, trace=False, **kw):
    x = np.ascontiguousarray(np.asarray(inputs["x"], dtype=np.float32))
    h = np.ascontiguousarray(
        np.asarray(inputs["hidden_state"], dtype=np.float32)
    )
    rnn_W = np.asarray(inputs["rnn_W"], dtype=np.float32)
    rnn_b = np.asarray(inputs["rnn_b"], dtype=np.float32)

    in_maps = make_in_maps(x, h, rnn_W, rnn_b)
    nc = get_program()
    res = run_bass_kernel_spmd(
        nc, in_maps, core_ids=list(range(N_CORES)), trace=trace, **kw
    )
    return gather_output(res.results), res


def kernel(**inputs) -> np.ndarray:
    out, _ = run(inputs)
    return out
